# revision 1
# baseline (speedup 1.0000x reference)
"""Trainium2 Bass kernel for a GPT transformer block (B=2, T=2048, E=1024, H=16).

Sharding: tensor-parallel over heads for attention (2 heads/core on 8 cores),
4 chunked ReduceScatters over tokens for the attention projection partial sums,
then token-parallel MLP (512 tokens/core with full FFN weights). Host gathers
per-core token slices into the full output.

Layout strategy (per core):
  - LN1 computed in [tok, E] layout (per-token scale/bias native), output cast
    to bf16 and DMA-transposed (xbar) to hT [E, tok] tiles for the qkv matmuls.
    LN gains/biases are folded into the qkv weights on the host.
  - qT/kT computed as [col, tok] (out = W.T @ hT); v computed as vT then
    DMA-transposed back to [tok, col], augmented with a ones column per head so
    the AV matmul also emits softmax denominators (row 64 of each y psum).
  - Attention computed transposed: scoresT[k, q] = kT.T @ qT per head into one
    two-bank psum tile, causal blocks only; one merged exp per k-block on ACT
    (no max subtraction -- scores are bounded by the input distribution);
    normalization via rank-1 outer-product divisor tiles.
  - proj -> fp32 partial sums -> chunked ReduceScatter -> residual -> LN2 ->
    fc/gelu/fc2 (bf16 matmuls, fp32 psum) -> residual -> out.
All matmul operands are bf16 (fp32 matmul runs at 1/4 rate); accumulation,
softmax statistics, LayerNorm and residuals stay fp32. ACT is used only for
Sqrt/Exp/Gelu in contiguous blocks (activation-table locality); psum drains and
affine applies run on DVE.
"""
import sys
if '/opt/trn_rl_repo' not in sys.path:
    sys.path.insert(0, '/opt/trn_rl_repo')

from contextlib import ExitStack

import numpy as np
import ml_dtypes

import concourse.bass as bass
import concourse.bacc as bacc
import concourse.tile as tile
import concourse.mybir as mybir

BF = mybir.dt.bfloat16
F32 = mybir.dt.float32
AF = mybir.ActivationFunctionType
MUL = mybir.AluOpType.mult
ADD = mybir.AluOpType.add

N_CORES = 8
B, T, E, H = 2, 2048, 1024, 16
HD = E // H                      # 64
NTOK = B * T                     # 4096
TOKC = NTOK // N_CORES           # 512 tokens per core for MLP
NCHUNK = 4                       # reduce-scatter chunks (1024 tokens each)
CHTOK = NTOK // NCHUNK           # 1024
SL = CHTOK // N_CORES            # 128 tokens per rank per chunk
EPS = 1e-5
SCALE = 1.0 / np.sqrt(HD)        # 0.125


def build_module(gelu_native=True, debug_taps=False, single_core=False, reps=1):
    nc = bacc.Bacc("TRN2", debug=False, num_devices=1 if single_core else N_CORES)
    dbg = {}

    # ---- I/O ----
    xbf_d = nc.dram_tensor("xbf", [NTOK, E], BF, kind="ExternalInput")
    xb2_d = nc.dram_tensor("xb2", [TOKC, E], F32, kind="ExternalInput")
    wq_d = nc.dram_tensor("wq", [128, 8, 128], BF, kind="ExternalInput")
    wk_d = nc.dram_tensor("wk", [128, 8, 128], BF, kind="ExternalInput")
    wv_d = nc.dram_tensor("wv", [128, 8, 128], BF, kind="ExternalInput")
    bq_d = nc.dram_tensor("bq", [128, 1], F32, kind="ExternalInput")
    bk_d = nc.dram_tensor("bk", [128, 1], F32, kind="ExternalInput")
    bvb_d = nc.dram_tensor("bvb", [128, 128], F32, kind="ExternalInput")
    wproj_d = nc.dram_tensor("wproj", [128, E], BF, kind="ExternalInput")
    trimask_d = nc.dram_tensor("trimask", [128, 128], BF, kind="ExternalInput")
    wfc_d = nc.dram_tensor("wfc", [32, 128, 8, 128], BF, kind="ExternalInput")
    bfcT_d = nc.dram_tensor("bfcT", [128, 32], F32, kind="ExternalInput")
    wfc2_d = nc.dram_tensor("wfc2", [32, 128, E], BF, kind="ExternalInput")
    bfc2b_d = nc.dram_tensor("bfc2b", [128, E], F32, kind="ExternalInput")
    out_d = nc.dram_tensor("out", [TOKC, E], F32, kind="ExternalOutput")
    if debug_taps:
        for nm, shp in [("dbg_qT", [128, T]), ("dbg_kT", [128, T]),
                        ("dbg_vsb", [128, 16 * 2 * 72]), ("dbg_e01", [128, 1024]),
                        ("dbg_y0", [128, 512]), ("dbg_rec", [65, 1024]),
                        ("dbg_yT", [128, 512]), ("dbg_pp", [128, E]),
                        ("dbg_x2", [128, E]), ("dbg_h4T", [128, 512])]:
            dbg[nm] = nc.dram_tensor(nm, shp, F32, kind="ExternalOutput")

    # internal DRAM for the collective
    cc_in = nc.dram_tensor("cc_in", [NTOK, E], F32)
    cc_outs = [nc.dram_tensor(f"cc_out{j}", [SL, E], F32) for j in range(NCHUNK)]
    RG = [list(range(N_CORES))]

    with ExitStack() as ctx:
        tc = ctx.enter_context(tile.TileContext(nc))

        # ---- persistent pools ----
        # PSUM budget (8 banks): psA tag x2 (1 bank each), s01 x2 (2 banks
        # each), y0 x1, y1 x1.  fc2 reuses: psA x2 + two s01 pairs + y0/y1.
        cst = ctx.enter_context(tc.tile_pool(name="cst", bufs=1))
        psA = ctx.enter_context(tc.tile_pool(name="psA", bufs=2, space="PSUM"))
        psS = ctx.enter_context(tc.tile_pool(name="psS", bufs=2, space="PSUM"))
        psY = ctx.enter_context(tc.tile_pool(name="psY", bufs=1, space="PSUM"))
        ppp = ctx.enter_context(tc.tile_pool(name="ppp", bufs=3))

        # ---- constants ----
        wq_sb = cst.tile([128, 8, 128], BF, tag="wq")
        wk_sb = cst.tile([128, 8, 128], BF, tag="wk")
        wv_sb = cst.tile([128, 8, 128], BF, tag="wv")
        nc.sync.dma_start(wq_sb[:], wq_d[:, :, :])
        nc.sync.dma_start(wk_sb[:], wk_d[:, :, :])
        nc.sync.dma_start(wv_sb[:], wv_d[:, :, :])
        bq_sb = cst.tile([128, 1], F32, tag="bq")
        bk_sb = cst.tile([128, 1], F32, tag="bk")
        bvb_sb = cst.tile([128, 128], F32, tag="bvb")
        nc.sync.dma_start(bq_sb[:], bq_d[:, :])
        nc.sync.dma_start(bk_sb[:], bk_d[:, :])
        nc.sync.dma_start(bvb_sb[:], bvb_d[:, :])
        wproj_sb = cst.tile([128, E], BF, tag="wproj")
        nc.sync.dma_start(wproj_sb[:], wproj_d[:, :])
        trimask_sb = cst.tile([128, 128], BF, tag="trimask")
        nc.sync.dma_start(trimask_sb[:], trimask_d[:, :])
        ones_row = cst.tile([65, 64], BF, tag="ones_row")   # D outer-product lhsT
        nc.gpsimd.memset(ones_row[:], 1.0)
        eps_sb = cst.tile([128, 1], F32, tag="eps")
        nc.gpsimd.memset(eps_sb[:], EPS)

        def tap(name, ap, pool_):
            if not debug_taps or name not in dbg:
                return
            t = pool_.tile([ap.shape[0], int(np.prod(ap.shape[1:]))], F32,
                           tag="dbgt", name=f"tap_{name}")
            nc.vector.tensor_copy(t[:ap.shape[0], :], ap)
            nc.sync.dma_start(dbg[name][0:ap.shape[0], :], t[:ap.shape[0], :])
            dbg.pop(name)

        def emit_body():
            with ExitStack() as p1:
                xp = p1.enter_context(tc.tile_pool(name="xp", bufs=3))
                stp = p1.enter_context(tc.tile_pool(name="stp", bufs=4))
                hp = p1.enter_context(tc.tile_pool(name="hp", bufs=3))
                hTp = p1.enter_context(tc.tile_pool(name="hTp", bufs=2))
                qkp = p1.enter_context(tc.tile_pool(name="qkp", bufs=2))
                vp = p1.enter_context(tc.tile_pool(name="vp", bufs=2))
                expp = p1.enter_context(tc.tile_pool(name="expp", bufs=4))
                srp = p1.enter_context(tc.tile_pool(name="srp", bufs=2))
                yTp = p1.enter_context(tc.tile_pool(name="yTp", bufs=2))
                ytmp = p1.enter_context(tc.tile_pool(name="ytmp", bufs=2))
                Dp = p1.enter_context(tc.tile_pool(name="Dp", bufs=2))
                dbgp = p1.enter_context(tc.tile_pool(name="dbgp", bufs=1)) if debug_taps else None

                # ---------- phase A: LN1 + transpose to hT (both batches) ----------
                hTs = []
                for b in range(B):
                    base = b * T
                    hT = hTp.tile([128, 8, T], BF, tag="hT", name=f"hT{b}")
                    hTs.append(hT)
                    for tt in range(T // 128):
                        x_sb = xp.tile([128, E], BF, tag="x")
                        nc.gpsimd.dma_start(x_sb[:], xbf_d[base + tt * 128: base + (tt + 1) * 128, :])
                        st = stp.tile([128, 2, 6], F32, tag="st")
                        nc.vector.bn_stats(st[:, 0, :], x_sb[:, 0:512])
                        nc.vector.bn_stats(st[:, 1, :], x_sb[:, 512:1024])
                        mv = stp.tile([128, 2], F32, tag="mv")
                        nc.vector.bn_aggr(mv[:], st[:])
                        rstd = stp.tile([128, 1], F32, tag="rstd")
                        nc.scalar.activation(rstd[:], mv[:, 1:2], AF.Sqrt, bias=eps_sb[:])
                        nc.vector.reciprocal(rstd[:], rstd[:])
                        nmr = stp.tile([128, 1], F32, tag="nmr")
                        nc.vector.scalar_tensor_tensor(nmr[:], mv[:, 0:1], -1.0, rstd[:],
                                                       op0=MUL, op1=MUL)
                        h_sb = hp.tile([128, E], BF, tag="h")
                        nc.vector.tensor_scalar(h_sb[:], x_sb[:], rstd[:], nmr[:],
                                                op0=MUL, op1=ADD)
                        for e in range(8):
                            nc.scalar.dma_start_transpose(
                                hT[:, e, tt * 128:(tt + 1) * 128],
                                h_sb[:, e * 128:(e + 1) * 128])

                # ---------- phase B: qkv + attention + proj, per batch ----------
                for b in range(B):
                    base = b * T
                    hT = hTs[b]
                    qT = qkp.tile([128, T], BF, tag="qT", name=f"qT{b}")
                    kT = qkp.tile([128, T], BF, tag="kT", name=f"kT{b}")
                    for w_sb, b_sb, dst in ((wq_sb, bq_sb, qT), (wk_sb, bk_sb, kT)):
                        for ch in range(T // 512):
                            ps = psA.tile([128, 512], F32, tag="psA", name="qkv_ps")
                            for e in range(8):
                                nc.tensor.matmul(ps[:], w_sb[:, e, :],
                                                 hT[:, e, ch * 512:(ch + 1) * 512],
                                                 start=(e == 0), stop=(e == 7))
                            nc.vector.tensor_scalar_add(dst[:, ch * 512:(ch + 1) * 512],
                                                        ps[:], b_sb[:])
                    # v computed directly in [tok, col] layout (stationary = hT slice):
                    # v_sb [128, kb, head, 72]: v cols 0:64, ones col at 64 (its own
                    # 16-byte SBUF line via the 72-col stride).
                    v_sb = vp.tile([128, 16, 2, 72], BF, tag="v")
                    for tt in range(T // 128):
                        ps = psA.tile([128, 512], F32, tag="psA", name="v_ps")
                        for e in range(8):
                            nc.tensor.matmul(ps[:, 0:128], hT[:, e, tt * 128:(tt + 1) * 128],
                                             wv_sb[:, e, :], start=(e == 0), stop=(e == 7))
                        nc.vector.tensor_add(
                            v_sb[:, tt, :, 0:64],
                            ps[:, 0:128].rearrange("p (g x) -> p g x", g=2),
                            bvb_sb[:].rearrange("p (g x) -> p g x", g=2))
                    nc.gpsimd.memset(v_sb[:, :, :, 64:65], 1.0)
                    if b == 0 and debug_taps:
                        tap("dbg_qT", qT[:], dbgp)
                        tap("dbg_kT", kT[:], dbgp)
                        tap("dbg_vsb", v_sb[:].rearrange("p a b c -> p (a b c)"), dbgp)

                    for qc in range(T // 512):
                        qs = qc * 512
                        y0 = psY.tile([128, 512], F32, tag="y0")
                        y1 = psY.tile([128, 512], F32, tag="y1")
                        nkb = 4 * qc + 4
                        for kb in range(nkb):
                            d = max(0, kb * 128 - qs)
                            s01 = psS.tile([128, 2, 512], F32, tag="s01")
                            nc.tensor.matmul(s01[:, 0, d:512],
                                             kT[0:64, kb * 128:(kb + 1) * 128],
                                             qT[0:64, qs + d:qs + 512])
                            nc.tensor.matmul(s01[:, 1, d:512],
                                             kT[64:128, kb * 128:(kb + 1) * 128],
                                             qT[64:128, qs + d:qs + 512])
                            e01 = expp.tile([128, 2, 512], BF, tag="e01")
                            nc.scalar.activation(e01[:, :, d:512], s01[:, :, d:512],
                                                 AF.Exp, scale=SCALE)
                            if kb * 128 >= qs:
                                nc.vector.tensor_mul(e01[:, 0, d:d + 128],
                                                     e01[:, 0, d:d + 128], trimask_sb[:])
                                nc.vector.tensor_mul(e01[:, 1, d:d + 128],
                                                     e01[:, 1, d:d + 128], trimask_sb[:])
                            if b == 0 and qc == 0 and kb == 0 and debug_taps:
                                tap("dbg_e01", e01[:].rearrange("p a b -> p (a b)"), dbgp)
                            st_, sp_ = (kb == 0), (kb == nkb - 1)
                            nc.tensor.matmul(y0[0:65, d:512], v_sb[:, kb, 0, 0:65],
                                             e01[:, 0, d:512], start=st_, stop=sp_)
                            nc.tensor.matmul(y1[0:65, d:512], v_sb[:, kb, 1, 0:65],
                                             e01[:, 1, d:512], start=st_, stop=sp_)
                        # softmax denominators (row 64 of y0/y1) -> divisor tiles
                        if b == 0 and qc == 0 and debug_taps:
                            tap("dbg_y0", y0[:], dbgp)
                        srec = srp.tile([65, 2, 512], F32, tag="srec")
                        recb = srp.tile([65, 2, 512], BF, tag="recb")
                        nc.vector.tensor_copy(srec[64:65, 0, :], y0[64:65, :])
                        nc.vector.tensor_copy(srec[64:65, 1, :], y1[64:65, :])
                        nc.vector.reciprocal(srec[64:65, :, :], srec[64:65, :, :])
                        nc.vector.tensor_copy(recb[64:65, :, :], srec[64:65, :, :])
                        D0 = psA.tile([128, 512], F32, tag="psA", name="D0")
                        D1 = psA.tile([128, 512], F32, tag="psA", name="D1")
                        nc.tensor.matmul(D0[0:64, :], ones_row[64:65, :], recb[64:65, 0, :],
                                         tile_position=(64, 0))
                        nc.tensor.matmul(D1[0:64, :], ones_row[64:65, :], recb[64:65, 1, :],
                                         tile_position=(64, 0))
                        D_sb = Dp.tile([64, 2, 512], BF, tag="D")
                        nc.vector.tensor_copy(D_sb[:, 0, :], D0[0:64, :])
                        nc.vector.tensor_copy(D_sb[:, 1, :], D1[0:64, :])
                        if b == 0 and qc == 0 and debug_taps:
                            tap("dbg_rec", srec[:].rearrange("p a b -> p (a b)"), dbgp)
                        yT = yTp.tile([128, 512], BF, tag="yT")
                        y1t = ytmp.tile([64, 512], BF, tag="y1t")
                        nc.vector.tensor_mul(yT[0:64, :], y0[0:64, :], D_sb[0:64, 0, :])
                        nc.vector.tensor_mul(y1t[:], y1[0:64, :], D_sb[0:64, 1, :])
                        nc.gpsimd.dma_start(yT[64:128, :], y1t[:, :])
                        if b == 0 and qc == 0 and debug_taps:
                            tap("dbg_yT", yT[:], dbgp)
                        # ---------- proj partial sums ----------
                        for mt in range(4):
                            pp0 = psA.tile([128, 512], F32, tag="psA", name="pp0")
                            pp1 = psA.tile([128, 512], F32, tag="psA", name="pp1")
                            nc.tensor.matmul(pp0[:], yT[:, mt * 128:(mt + 1) * 128],
                                             wproj_sb[:, 0:512])
                            nc.tensor.matmul(pp1[:], yT[:, mt * 128:(mt + 1) * 128],
                                             wproj_sb[:, 512:1024])
                            pp_sb = ppp.tile([128, E], F32, tag="pp")
                            nc.vector.tensor_copy(pp_sb[:, 0:512], pp0[:])
                            nc.vector.tensor_copy(pp_sb[:, 512:1024], pp1[:])
                            row = base + qs + mt * 128
                            nc.sync.dma_start(cc_in[row:row + 128, :], pp_sb[:])
                            if row == 0 and debug_taps:
                                tap("dbg_pp", pp_sb[:], dbgp)
                        # fire the reduce-scatter for each completed 1024-token chunk
                        if qc % 2 == 1:
                            j = b * 2 + qc // 2
                            if single_core:
                                t_ = ppp.tile([SL, E], F32, tag="pp", name="rsfake")
                                nc.sync.dma_start(t_[0:SL, :], cc_in[j * CHTOK:j * CHTOK + SL, :])
                                nc.sync.dma_start(cc_outs[j][:, :], t_[0:SL, :])
                            else:
                                nc.gpsimd.collective_compute(
                                    "ReduceScatter", mybir.AluOpType.add,
                                    replica_groups=RG,
                                    ins=[cc_in[j * CHTOK:(j + 1) * CHTOK, :]],
                                    outs=[cc_outs[j][:, :]])

            # ---------- phase C: MLP (token-parallel, full weights) ----------
            with ExitStack() as p2:
                x2p = p2.enter_context(tc.tile_pool(name="x2p", bufs=1))
                st2 = p2.enter_context(tc.tile_pool(name="st2", bufs=4))
                h2p = p2.enter_context(tc.tile_pool(name="h2p", bufs=2))
                h2Tp = p2.enter_context(tc.tile_pool(name="h2Tp", bufs=1))
                h4Tp = p2.enter_context(tc.tile_pool(name="h4Tp", bufs=1))
                wfcp = p2.enter_context(tc.tile_pool(name="wfcp", bufs=3))
                wf2p = p2.enter_context(tc.tile_pool(name="wf2p", bufs=3))
                outp = p2.enter_context(tc.tile_pool(name="outp", bufs=2))
                gwp = p2.enter_context(tc.tile_pool(name="gwp", bufs=2))
                dbgp2 = p2.enter_context(tc.tile_pool(name="dbgp2", bufs=2)) if debug_taps else None
                cst2 = p2.enter_context(tc.tile_pool(name="cst2", bufs=1))

                bfcT_sb = cst2.tile([128, 32], F32, tag="bfcT")
                nc.sync.dma_start(bfcT_sb[:], bfcT_d[:, :])
                bfc2b_sb = cst2.tile([128, E], F32, tag="bfc2b")
                nc.sync.dma_start(bfc2b_sb[:], bfc2b_d[:, :])

                h2T = h2Tp.tile([128, 8, TOKC], BF, tag="h2T")
                x2_tiles = []
                for mt in range(4):
                    x2 = x2p.tile([128, E], F32, tag=f"x2_{mt}", name=f"x2_{mt}")
                    x2_tiles.append(x2)
                    xr = x2p.tile([128, E], F32, tag="xr")
                    nc.sync.dma_start(x2[:], cc_outs[mt][:, :])
                    nc.sync.dma_start(xr[:], xb2_d[mt * 128:(mt + 1) * 128, :])
                    nc.vector.tensor_add(x2[:], x2[:], xr[:])
                    st = st2.tile([128, 2, 6], F32, tag="st")
                    nc.vector.bn_stats(st[:, 0, :], x2[:, 0:512])
                    nc.vector.bn_stats(st[:, 1, :], x2[:, 512:1024])
                    mv = st2.tile([128, 2], F32, tag="mv")
                    nc.vector.bn_aggr(mv[:], st[:])
                    rstd = st2.tile([128, 1], F32, tag="rstd")
                    nc.scalar.activation(rstd[:], mv[:, 1:2], AF.Sqrt, bias=eps_sb[:])
                    nc.vector.reciprocal(rstd[:], rstd[:])
                    nmr = st2.tile([128, 1], F32, tag="nmr")
                    nc.vector.scalar_tensor_tensor(nmr[:], mv[:, 0:1], -1.0, rstd[:],
                                                   op0=MUL, op1=MUL)
                    h2 = h2p.tile([128, E], BF, tag="h2")
                    nc.vector.tensor_scalar(h2[:], x2[:], rstd[:], nmr[:],
                                            op0=MUL, op1=ADD)
                    for e in range(8):
                        nc.scalar.dma_start_transpose(
                            h2T[:, e, mt * 128:(mt + 1) * 128],
                            h2[:, e * 128:(e + 1) * 128])

                if debug_taps:
                    tap("dbg_x2", x2_tiles[0][:], dbgp2)
                # fc + gelu -> h4T
                h4T = h4Tp.tile([128, 32, TOKC], BF, tag="h4T")
                for m in range(32):
                    wfc_sb = wfcp.tile([128, 8, 128], BF, tag="wfc")
                    nc.sync.dma_start(wfc_sb[:], wfc_d[m, :, :, :])
                    h3 = psA.tile([128, 512], F32, tag="psA", name="h3")
                    for e in range(8):
                        nc.tensor.matmul(h3[:], wfc_sb[:, e, :], h2T[:, e, :],
                                         start=(e == 0), stop=(e == 7))
                    if gelu_native:
                        nc.scalar.activation(h4T[:, m, :], h3[:], AF.Gelu_apprx_tanh,
                                             bias=bfcT_sb[:, m:m + 1])
                    else:
                        # tanh-gelu built from sim-supported ops:
                        #   u = c*(h3b + 0.044715*h3b^3); h4 = h3b*(0.5+0.5*tanh(u))
                        h3b = gwp.tile([128, 512], F32, tag="h3b")
                        nc.vector.tensor_scalar_add(h3b[:], h3[:], bfcT_sb[:, m:m + 1])
                        sq = gwp.tile([128, 512], F32, tag="sq")
                        nc.vector.tensor_mul(sq[:], h3b[:], h3b[:])
                        nc.vector.tensor_scalar(sq[:], sq[:], 0.044715, 1.0,
                                                op0=MUL, op1=ADD)
                        nc.vector.tensor_mul(sq[:], sq[:], h3b[:])
                        th = gwp.tile([128, 512], F32, tag="th")
                        nc.scalar.activation(th[:], sq[:], AF.Tanh,
                                             scale=float(np.sqrt(2.0 / np.pi)))
                        nc.vector.tensor_scalar(th[:], th[:], 0.5, 0.5, op0=MUL, op1=ADD)
                        nc.vector.tensor_mul(h4T[:, m, :], th[:], h3b[:])

                if debug_taps:
                    tap("dbg_h4T", h4T[:, 0, :], dbgp2)
                # fc2: all 8 psum banks accumulate across the single weight stream
                y2 = [
                    (psA.tile([128, 512], F32, tag="psA", name="y2a0"),
                     psA.tile([128, 512], F32, tag="psA", name="y2b0")),
                    (psS.tile([128, 2, 512], F32, tag="s01", name="y2p1"),),
                    (psS.tile([128, 2, 512], F32, tag="s01", name="y2p2"),),
                    (psY.tile([128, 512], F32, tag="y0", name="y2a3"),
                     psY.tile([128, 512], F32, tag="y1", name="y2b3")),
                ]

                def y2ap(mt, half):
                    tt = y2[mt]
                    if len(tt) == 1:
                        return tt[0][:, half, :]
                    return tt[half][:]

                for k in range(32):
                    wf2 = wf2p.tile([128, E], BF, tag="wf2")
                    nc.gpsimd.dma_start(wf2[:], wfc2_d[k, :, :])
                    for mt in range(4):
                        nc.tensor.matmul(y2ap(mt, 0), h4T[:, k, mt * 128:(mt + 1) * 128],
                                         wf2[:, 0:512], start=(k == 0), stop=(k == 31))
                        nc.tensor.matmul(y2ap(mt, 1), h4T[:, k, mt * 128:(mt + 1) * 128],
                                         wf2[:, 512:1024], start=(k == 0), stop=(k == 31))
                for mt in range(4):
                    o = outp.tile([128, E], F32, tag="o")
                    nc.vector.tensor_add(o[:, 0:512], y2ap(mt, 0), x2_tiles[mt][:, 0:512])
                    nc.vector.tensor_add(o[:, 512:1024], y2ap(mt, 1), x2_tiles[mt][:, 512:1024])
                    nc.vector.tensor_add(o[:], o[:], bfc2b_sb[:])
                    nc.sync.dma_start(out_d[mt * 128:(mt + 1) * 128, :], o[:])


        for _rep in range(reps):
            emit_body()

    nc.compile()
    return nc


def prep_inputs(x, ln1_g, ln1_b, w_attn, b_attn, w_proj, b_proj,
                ln2_g, ln2_b, w_fc, b_fc, w_fc2, b_fc2):
    """Host-side prep: fold LN affine into weights, slice per core, cast bf16."""
    bf16 = ml_dtypes.bfloat16
    x_flat = np.asarray(x, np.float32).reshape(NTOK, E)
    w_attn = np.asarray(w_attn, np.float32)
    ln1_g = np.asarray(ln1_g, np.float32)
    ln1_b = np.asarray(ln1_b, np.float32)
    ln2_g = np.asarray(ln2_g, np.float32)
    ln2_b = np.asarray(ln2_b, np.float32)
    wa_eff = ln1_g[:, None] * w_attn
    ba_eff = ln1_b @ w_attn + np.asarray(b_attn, np.float32)
    wf_eff = ln2_g[:, None] * np.asarray(w_fc, np.float32)
    bf_eff = ln2_b @ np.asarray(w_fc, np.float32) + np.asarray(b_fc, np.float32)

    tri = (np.arange(128)[None, :] >= np.arange(128)[:, None]).astype(bf16)
    wfc_arr = np.ascontiguousarray(wf_eff.reshape(8, 128, 32, 128).transpose(2, 1, 0, 3)).astype(bf16)
    bfcT = bf_eff.reshape(32, 128).T.astype(np.float32).copy()
    wfc2_arr = np.asarray(w_fc2, np.float32).reshape(32, 128, E).astype(bf16)
    bfc2b = np.broadcast_to(np.asarray(b_fc2, np.float32), (128, E)).copy()
    xbf = x_flat.astype(bf16)

    in_maps = []
    for r in range(N_CORES):
        qc_ = slice(128 * r, 128 * r + 128)
        kc_ = slice(E + 128 * r, E + 128 * r + 128)
        vc_ = slice(2 * E + 128 * r, 2 * E + 128 * r + 128)
        xb2 = np.concatenate(
            [x_flat[j * CHTOK + r * SL: j * CHTOK + (r + 1) * SL] for j in range(NCHUNK)],
            axis=0) + np.asarray(b_proj, np.float32)
        in_maps.append({
            "xbf": xbf,
            "xb2": xb2.astype(np.float32),
            "wq": np.ascontiguousarray(wa_eff[:, qc_].reshape(8, 128, 128).transpose(1, 0, 2)).astype(bf16),
            "wk": np.ascontiguousarray(wa_eff[:, kc_].reshape(8, 128, 128).transpose(1, 0, 2)).astype(bf16),
            "wv": np.ascontiguousarray(wa_eff[:, vc_].reshape(8, 128, 128).transpose(1, 0, 2)).astype(bf16),
            "bq": ba_eff[qc_].reshape(128, 1).astype(np.float32),
            "bk": ba_eff[kc_].reshape(128, 1).astype(np.float32),
            "bvb": np.tile(ba_eff[vc_].astype(np.float32), (128, 1)),
            "wproj": np.asarray(w_proj, np.float32)[128 * r:128 * r + 128, :].astype(bf16),
            "trimask": tri,
            "wfc": wfc_arr,
            "bfcT": bfcT,
            "wfc2": wfc2_arr,
            "bfc2b": bfc2b,
        })
    return in_maps


def gather_output(results):
    out_flat = np.empty((NTOK, E), np.float32)
    for r in range(N_CORES):
        o = results[r]["out"]
        for j in range(NCHUNK):
            out_flat[j * CHTOK + r * SL: j * CHTOK + (r + 1) * SL] = o[j * SL:(j + 1) * SL]
    return out_flat.reshape(B, T, E)


_CACHE = {}


def _get_runner():
    if "runner" in _CACHE:
        return _CACHE["runner"]
    import jax
    from jax.sharding import Mesh, PartitionSpec, NamedSharding
    from jax.experimental.shard_map import shard_map
    from concourse.bass2jax import _bass_exec_p, install_neuronx_cc_hook, partition_id_tensor

    nc = build_module()
    install_neuronx_cc_hook()
    partition_name = nc.partition_id_tensor.name if nc.partition_id_tensor else None
    in_names, out_names, out_avals = [], [], []
    for alloc in nc.m.functions[0].allocations:
        if not isinstance(alloc, mybir.MemoryLocationSet):
            continue
        name = alloc.memorylocations[0].name
        if alloc.kind == "ExternalInput":
            if name != partition_name:
                in_names.append(name)
        elif alloc.kind == "ExternalOutput":
            out_names.append(name)
            out_avals.append(jax.core.ShapedArray(
                tuple(alloc.tensor_shape), mybir.dt.np(alloc.dtype)))
    all_in = in_names + out_names + ([partition_name] if partition_name else [])

    def _body(*args):
        operands = list(args)
        if partition_name is not None:
            operands.append(partition_id_tensor())
        return tuple(_bass_exec_p.bind(
            *operands, out_avals=tuple(out_avals), in_names=tuple(all_in),
            out_names=tuple(out_names), lowering_input_output_aliases=(),
            sim_require_finite=True, sim_require_nnan=True, nc=nc))

    devices = jax.devices()[:N_CORES]
    mesh = Mesh(np.asarray(devices), ("core",))
    n_io = len(in_names) + len(out_names)
    fn = jax.jit(
        shard_map(_body, mesh=mesh, in_specs=(PartitionSpec("core"),) * n_io,
                  out_specs=(PartitionSpec("core"),) * len(out_names),
                  check_rep=False),
        keep_unused=True)
    sharding = NamedSharding(mesh, PartitionSpec("core"))
    _CACHE["runner"] = (fn, in_names, out_names, out_avals, sharding)
    return _CACHE["runner"]


def run_device(in_maps):
    import jax
    fn, in_names, out_names, out_avals, sharding = _get_runner()
    concat_in = [
        np.concatenate([np.asarray(in_maps[c][n]) for c in range(N_CORES)], axis=0)
        for n in in_names]
    concat_zero = [np.zeros((N_CORES * a.shape[0], *a.shape[1:]), a.dtype)
                   for a in out_avals]
    args = [jax.device_put(a, sharding) for a in concat_in + concat_zero]
    outs = fn(*args)
    jax.block_until_ready(outs)
    return [
        {n: np.asarray(outs[i]).reshape(N_CORES, *out_avals[i].shape)[c]
         for i, n in enumerate(out_names)}
        for c in range(N_CORES)], args, fn


def kernel(**inputs):
    in_maps = prep_inputs(**inputs)
    results, _, _ = run_device(in_maps)
    return gather_output(results).astype(np.float32)



# revision 2
# speedup vs baseline: 11.9664x; 11.9664x over previous
"""Trainium2 Bass kernel v2 for a GPT transformer block (B=2, T=2048, E=1024, H=16).

Sharding: cores 0-3 handle batch 0, cores 4-7 batch 1. Within its batch group,
core r owns 4 heads (qkv columns [256*(r%4), 256*(r%4)+256)) for attention and
512 tokens (4 strided 128-blocks, one per 512-token chunk) for proj/MLP.
Attention outputs are exchanged with a small bf16 AllToAll per 512-token chunk
(4-rank groups), after which proj/LN2/MLP run token-parallel with full weights.

Fully transposed dataflow: activations live as [feature-partition, token-free]
end to end, so NO DMA transposes exist anywhere:
  - xT loaded straight from DRAM with 8 big transposing DMAs (xbar).
  - LN1/LN2 computed in transposed space: col-sums via ones-vector matmuls on
    PE, rowvec stats math on DVE/ACT (Rsqrt), per-token scale/shift broadcast
    back to [128, tok] with rank-1 ones outer-products on PE, applied by DVE.
  - attention is transposed (scoresT = kT.T @ qT) with merged exp, ones-column
    denominators, and rank-1 divisor tiles.
  - proj and fc2 compute transposed outputs (E on partitions); the host
    transposes the final [8, 128, 512] per-core output back.
All matmul operands bf16 (fp32 accumulation in PSUM); LN rowvec stats and the
residual accumulator stay fp32. The owned-token residual slice (x + b_proj,
transposed) comes from the host as `xo` so the SPMD program needs no
core-dependent addressing.
"""
import sys
if '/opt/trn_rl_repo' not in sys.path:
    sys.path.insert(0, '/opt/trn_rl_repo')

from contextlib import ExitStack

import numpy as np
import ml_dtypes

import concourse.bass as bass
import concourse.bacc as bacc
import concourse.tile as tile
import concourse.mybir as mybir

BF = mybir.dt.bfloat16
F32 = mybir.dt.float32
AF = mybir.ActivationFunctionType
MUL = mybir.AluOpType.mult
ADD = mybir.AluOpType.add
SUB = mybir.AluOpType.subtract

N_CORES = 8
B, T, E, H = 2, 2048, 1024, 16
HD = E // H                      # 64
GSZ = N_CORES // B               # 4 cores per batch group
COLS = 256                       # qkv columns per core (4 heads)
TOKC = T // GSZ                  # 512 tokens per core (proj/MLP)
EPS = 1e-5
SCALE = 1.0 / np.sqrt(HD)        # 0.125
RG = [[0, 1, 2, 3, 4, 5, 6, 7]]


def build_module(single_core=False, reps=1):
    nc = bacc.Bacc("TRN2", debug=False, num_devices=1 if single_core else N_CORES)

    # ---- I/O ----
    xT_d = nc.dram_tensor("xT", [128, 8, T], BF, kind="ExternalInput")
    xo_d = nc.dram_tensor("xo", [128, 8, TOKC], F32, kind="ExternalInput")
    wq_d = nc.dram_tensor("wq", [128, 2, 8, 128], BF, kind="ExternalInput")
    wk_d = nc.dram_tensor("wk", [128, 2, 8, 128], BF, kind="ExternalInput")
    wv_d = nc.dram_tensor("wv", [128, 8, 256], BF, kind="ExternalInput")
    bq_d = nc.dram_tensor("bq", [128, 2], F32, kind="ExternalInput")
    bk_d = nc.dram_tensor("bk", [128, 2], F32, kind="ExternalInput")
    bvb_d = nc.dram_tensor("bvb", [128, 256], F32, kind="ExternalInput")
    wproj_d = nc.dram_tensor("wproj", [128, 8, 8, 128], BF, kind="ExternalInput")
    trimask_d = nc.dram_tensor("trimask", [128, 128], BF, kind="ExternalInput")
    wfc_d = nc.dram_tensor("wfc", [32, 128, 8, 128], BF, kind="ExternalInput")
    bfcT_d = nc.dram_tensor("bfcT", [128, 32], F32, kind="ExternalInput")
    wfc2_d = nc.dram_tensor("wfc2", [32, 128, 8, 128], BF, kind="ExternalInput")
    bfc2T_d = nc.dram_tensor("bfc2T", [128, 8], F32, kind="ExternalInput")
    out_d = nc.dram_tensor("out", [8, 128, TOKC], F32, kind="ExternalOutput")

    # internal DRAM for the 8-rank AllToAll (2 rounds; parity dest mapping)
    a2a_in = [nc.dram_tensor(f"a2a_in{j}", [16, 128, 128], BF) for j in range(2)]
    a2a_out = [nc.dram_tensor(f"a2a_out{j}", [16, 128, 128], BF) for j in range(2)]

    with ExitStack() as ctx:
        tc = ctx.enter_context(tile.TileContext(nc))

        # ---- persistent constants ----
        cst = ctx.enter_context(tc.tile_pool(name="cst", bufs=1))
        ones_col = cst.tile([128, 1], BF, tag="ones_col")    # col-sum lhsT
        nc.gpsimd.memset(ones_col[:], 1.0)
        ones_bc = cst.tile([1, 128], BF, tag="ones_bc")      # rowvec-bcast lhsT
        nc.gpsimd.memset(ones_bc[:], 1.0)
        ones_row = cst.tile([65, 64], BF, tag="ones_row")    # divisor outer lhsT
        nc.gpsimd.memset(ones_row[:], 1.0)
        eps_sb = cst.tile([128, 1], F32, tag="eps")
        nc.gpsimd.memset(eps_sb[:], EPS)
        wq_sb = cst.tile([128, 2, 8, 128], BF, tag="wq")
        wk_sb = cst.tile([128, 2, 8, 128], BF, tag="wk")
        wv_sb = cst.tile([128, 8, 256], BF, tag="wv")
        nc.gpsimd.dma_start(wq_sb[:], wq_d[:, :, :, :])
        nc.gpsimd.dma_start(wk_sb[:], wk_d[:, :, :, :])
        nc.gpsimd.dma_start(wv_sb[:], wv_d[:, :, :])
        bq_sb = cst.tile([128, 2], F32, tag="bq")
        bk_sb = cst.tile([128, 2], F32, tag="bk")
        bvb_sb = cst.tile([128, 256], F32, tag="bvb")
        nc.gpsimd.dma_start(bq_sb[:], bq_d[:, :])
        nc.gpsimd.dma_start(bk_sb[:], bk_d[:, :])
        nc.gpsimd.dma_start(bvb_sb[:], bvb_d[:, :])
        wproj_sb = cst.tile([128, 8, 8, 128], BF, tag="wproj")
        nc.gpsimd.dma_start(wproj_sb[:], wproj_d[:, :, :, :])
        trimask_sb = cst.tile([128, 128], BF, tag="trimask")
        nc.gpsimd.dma_start(trimask_sb[:], trimask_d[:, :])
        bfcT_sb = cst.tile([128, 32], F32, tag="bfcT")
        nc.gpsimd.dma_start(bfcT_sb[:], bfcT_d[:, :])
        bfc2T_sb = cst.tile([128, 8], F32, tag="bfc2T")
        nc.gpsimd.dma_start(bfc2T_sb[:], bfc2T_d[:, :])
        xo_sb = cst.tile([128, 8, TOKC], F32, tag="xo")
        nc.gpsimd.dma_start(xo_sb[:], xo_d[:, :, :])

        def ln_transposed(dst, xt, nch, psP, psB, sqp, rvp, tmpl, csz=512):
            """dst = LN(xt) per token chunk, fully in transposed space.
            Stats via ones-matmul col sums; -mu and std broadcast with rank-1
            outer products; the reciprocal runs full-width on the broadcast.
            Squares and the shift-adds run on the Pool engine (DVE relief)."""
            for c in range(nch):
                sl = slice(c * csz, (c + 1) * csz)
                sx = psP.tile([1, csz], F32, tag="sx", name="sx", bufs=2)
                sq = psP.tile([1, csz], F32, tag="sq", name="sq", bufs=2)
                for e in range(8):
                    nc.tensor.matmul(sx[:], ones_col[:], xt[:, e, sl],
                                     start=(e == 0), stop=(e == 7))
                for e in range(8):
                    sqt = sqp.tile([128, csz], BF, tag="sqt", bufs=3)
                    nc.vector.tensor_mul(sqt[:], xt[:, e, sl], xt[:, e, sl])
                    nc.tensor.matmul(sq[:], ones_col[:], sqt[:],
                                     start=(e == 0), stop=(e == 7))
                negmu = rvp.tile([1, csz], F32, tag="negmu", bufs=2)
                nc.vector.tensor_scalar_mul(negmu[:], sx[:], -1.0 / E)
                var = rvp.tile([1, csz], F32, tag="rv", name="var", bufs=2)
                nc.vector.tensor_mul(var[:], negmu[:], negmu[:])
                nc.vector.scalar_tensor_tensor(var[:], sq[:], 1.0 / E, var[:],
                                               op0=MUL, op1=SUB)
                nc.scalar.activation(var[:], var[:], AF.Sqrt, bias=eps_sb[0:1])
                stdb = rvp.tile([1, csz], BF, tag="stdb", bufs=2)
                nc.vector.tensor_copy(stdb[:], var[:])
                mub = rvp.tile([1, csz], BF, tag="mub", bufs=2)
                nc.vector.tensor_copy(mub[:], negmu[:])
                M = psB.tile([128, csz], F32, tag="bcM", name="bcM")
                S = psB.tile([128, csz], F32, tag="bcS", name="bcS")
                nc.tensor.matmul(M[:], ones_bc[:], mub[:])
                nc.tensor.matmul(S[:], ones_bc[:], stdb[:])
                Rf = tmpl.tile([128, csz], F32, tag="Rf", bufs=2)
                nc.vector.reciprocal_approx_fast(Rf[:], S[:])
                for e in range(8):
                    t = tmpl.tile([128, csz], BF, tag="lnt", bufs=3)
                    nc.vector.tensor_add(t[:], M[:], xt[:, e, sl])
                    nc.vector.tensor_mul(dst[:, e, sl], t[:], Rf[:])

        def emit_body():
            with ExitStack() as p0:
                x2Tp = p0.enter_context(tc.tile_pool(name="x2Tp", bufs=1))
                x2T = x2Tp.tile([128, 8, TOKC], BF, tag="x2T")
                h2T = x2Tp.tile([128, 8, TOKC], BF, tag="h2T")

                with ExitStack() as pAD:
                    xTp = pAD.enter_context(tc.tile_pool(name="xTp", bufs=1))
                    # ------ phase A: transposing load + LN1 (transposed) ----
                    xT = xTp.tile([128, 8, T], BF, tag="xT")
                    for c in range(4):
                        for e in range(8):
                            q = nc.sync if e % 2 == 0 else nc.scalar
                            q.dma_start(xT[:, e, c * 512:(c + 1) * 512],
                                        xT_d[:, e, c * 512:(c + 1) * 512])
                    with ExitStack() as pABC:
                        qkp = pABC.enter_context(tc.tile_pool(name="qkp", bufs=1))
                        vp = pABC.enter_context(tc.tile_pool(name="vp", bufs=1))
                        with ExitStack() as pAB:
                            hTp = pAB.enter_context(tc.tile_pool(name="hTp", bufs=1))
                            hT = hTp.tile([128, 8, T], BF, tag="hT")
                            with ExitStack() as pA:
                                psP = pA.enter_context(tc.tile_pool(name="psP", bufs=1, space="PSUM"))
                                psB = pA.enter_context(tc.tile_pool(name="psB", bufs=1, space="PSUM"))
                                sqp = pA.enter_context(tc.tile_pool(name="sqp", bufs=1))
                                rvp = pA.enter_context(tc.tile_pool(name="rvp", bufs=1))
                                tmpl = pA.enter_context(tc.tile_pool(name="tmpl", bufs=1))
                                ln_transposed(hT, xT, 4, psP, psB, sqp, rvp, tmpl)

                            # ------ phase B: qkv ----------
                            qTs, kTs = [], []
                            with ExitStack() as pQ:
                                psQ = pQ.enter_context(tc.tile_pool(name="psQ", bufs=2, space="PSUM"))
                                for hp in range(2):
                                    qT = qkp.tile([128, T], BF, tag=f"qT{hp}", name=f"qT{hp}")
                                    kT = qkp.tile([128, T], BF, tag=f"kT{hp}", name=f"kT{hp}")
                                    qTs.append(qT); kTs.append(kT)
                                    for w_sb, b_sb, dst in ((wq_sb, bq_sb, qT),
                                                            (wk_sb, bk_sb, kT)):
                                        for ch in range(T // 512):
                                            ps = psQ.tile([128, 512], F32, tag="psQ",
                                                          name="qk_ps")
                                            for e in range(8):
                                                nc.tensor.matmul(
                                                    ps[:], w_sb[:, hp, e, :],
                                                    hT[:, e, ch * 512:(ch + 1) * 512],
                                                    start=(e == 0), stop=(e == 7))
                                            nc.vector.tensor_scalar_add(
                                                dst[:, ch * 512:(ch + 1) * 512],
                                                ps[:], b_sb[:, hp:hp + 1])
                                v_sb = vp.tile([128, 16, 4, 72], BF, tag="v")
                                for tt in range(T // 128):
                                    ps = psQ.tile([128, 512], F32, tag="psQ", name="v_ps")
                                    for e in range(8):
                                        nc.tensor.matmul(
                                            ps[:, 0:256], hT[:, e, tt * 128:(tt + 1) * 128],
                                            wv_sb[:, e, :], start=(e == 0), stop=(e == 7))
                                    nc.vector.tensor_add(
                                        v_sb[:, tt, :, 0:64],
                                        ps[:, 0:256].rearrange("p (g x) -> p g x", g=4),
                                        bvb_sb[:].rearrange("p (g x) -> p g x", g=4))
                                nc.gpsimd.memset(v_sb[:, :, :, 64:65], 1.0)

                        # ------ phase C: attention + A2A ----------
                        with ExitStack() as pC:
                            psS = pC.enter_context(tc.tile_pool(name="psS", bufs=2, space="PSUM"))
                            psY = pC.enter_context(tc.tile_pool(name="psY", bufs=1, space="PSUM"))
                            psD = pC.enter_context(tc.tile_pool(name="psD", bufs=1, space="PSUM"))
                            expp = pC.enter_context(tc.tile_pool(name="expp", bufs=4))
                            srp = pC.enter_context(tc.tile_pool(name="srp", bufs=2))
                            yTp = pC.enter_context(tc.tile_pool(name="yTp", bufs=4))

                            for qc in range(4):
                                qs = qc * 512
                                nkb = 4 * qc + 4
                                for hp in range(2):
                                    qT, kT = qTs[hp], kTs[hp]
                                    y0 = psY.tile([128, 512], F32, tag="y0")
                                    y1 = psY.tile([128, 512], F32, tag="y1")
                                    for kb in range(nkb):
                                        d = max(0, kb * 128 - qs)
                                        s01 = psS.tile([128, 2, 512], F32, tag="s01")
                                        nc.tensor.matmul(s01[:, 0, d:512],
                                                         kT[0:64, kb * 128:(kb + 1) * 128],
                                                         qT[0:64, qs + d:qs + 512])
                                        nc.tensor.matmul(s01[:, 1, d:512],
                                                         kT[64:128, kb * 128:(kb + 1) * 128],
                                                         qT[64:128, qs + d:qs + 512])
                                        e01 = expp.tile([128, 2, 512], BF, tag="e01")
                                        nc.scalar.activation(e01[:, :, d:512],
                                                             s01[:, :, d:512],
                                                             AF.Exp, scale=SCALE)
                                        if kb * 128 >= qs:
                                            nc.vector.tensor_mul(
                                                e01[:, 0, d:d + 128],
                                                e01[:, 0, d:d + 128], trimask_sb[:])
                                            nc.vector.tensor_mul(
                                                e01[:, 1, d:d + 128],
                                                e01[:, 1, d:d + 128], trimask_sb[:])
                                        st_, sp_ = (kb == 0), (kb == nkb - 1)
                                        nc.tensor.matmul(y0[0:65, d:512],
                                                         v_sb[:, kb, 2 * hp, 0:65],
                                                         e01[:, 0, d:512],
                                                         start=st_, stop=sp_)
                                        nc.tensor.matmul(y1[0:65, d:512],
                                                         v_sb[:, kb, 2 * hp + 1, 0:65],
                                                         e01[:, 1, d:512],
                                                         start=st_, stop=sp_)
                                    ysb = srp.tile([65, 2, 512], F32, tag="ysb")
                                    nc.vector.tensor_copy(ysb[:, 0, :], y0[0:65, :])
                                    nc.vector.tensor_copy(ysb[:, 1, :], y1[0:65, :])
                                    denb = srp.tile([65, 2, 512], BF, tag="denb")
                                    nc.vector.tensor_copy(denb[64:65, :, :],
                                                          ysb[64:65, :, :])
                                    D01 = psD.tile([128, 2, 512], F32, tag="psD",
                                                   name="D01")
                                    nc.tensor.matmul(D01[0:64, 0, :], ones_row[64:65, :],
                                                     denb[64:65, 0, :],
                                                     tile_position=(64, 0))
                                    nc.tensor.matmul(D01[0:64, 1, :], ones_row[64:65, :],
                                                     denb[64:65, 1, :],
                                                     tile_position=(64, 0))
                                    Drec = srp.tile([64, 2, 512], F32, tag="Drec")
                                    nc.vector.reciprocal_approx_fast(Drec[:],
                                                                     D01[0:64, :, :])
                                    for h in range(2):
                                        yT = yTp.tile([64, 4, 128], BF, tag="yT",
                                                      name=f"yT{qc}{hp}{h}")
                                        nc.vector.tensor_mul(
                                            yT[:],
                                            ysb[0:64, h, :].rearrange("p (a b) -> p a b", b=128),
                                            Drec[:, h, :].rearrange("p (a b) -> p a b", b=128))
                                        for s in range(4):
                                            sh = 2 * (2 * s + qc % 2) + hp
                                            nc.gpsimd.dma_start(
                                                a2a_in[qc // 2][sh,
                                                                64 * h:64 * h + 64, :],
                                                yT[:, s, :])
                                if qc % 2 == 1:
                                    p = qc // 2
                                    if single_core:
                                        t_ = yTp.tile([128, 16, 128], BF, tag="a2af",
                                                      name="a2af")
                                        nc.sync.dma_start(
                                            t_[:], a2a_in[p].rearrange("a p b -> p a b"))
                                        nc.sync.dma_start(
                                            a2a_out[p].rearrange("a p b -> p a b"), t_[:])
                                    else:
                                        nc.gpsimd.collective_compute(
                                            "AllToAll", mybir.AluOpType.bypass,
                                            replica_groups=RG,
                                            ins=[a2a_in[p][:, :, :]],
                                            outs=[a2a_out[p][:, :, :]])

                    # ------ phase D: proj + residual + LN2, per A2A round -----
                    with ExitStack() as pD:
                        yfp = pD.enter_context(tc.tile_pool(name="yfp", bufs=1))
                        psJ = pD.enter_context(tc.tile_pool(name="psJ", bufs=2, space="PSUM"))
                        psP2 = pD.enter_context(tc.tile_pool(name="psP2", bufs=1, space="PSUM"))
                        psB2 = pD.enter_context(tc.tile_pool(name="psB2", bufs=1, space="PSUM"))
                        sqp2 = pD.enter_context(tc.tile_pool(name="sqp2", bufs=1))
                        rvp2 = pD.enter_context(tc.tile_pool(name="rvp2", bufs=1))
                        tmpl2 = pD.enter_context(tc.tile_pool(name="tmpl2", bufs=1))
                        yTfull = yfp.tile([128, 8, TOKC], BF, tag="yTfull")
                        for p in range(2):
                            po = p * 256
                            for b in range(2):
                                blk = 2 * p + b
                                for eb in range(8):
                                    q = nc.gpsimd if eb % 2 == 0 else nc.sync
                                    q.dma_start(
                                        yTfull[:, eb, blk * 128:(blk + 1) * 128],
                                        a2a_out[p][8 * b + eb, :, :])
                            for eb in range(8):
                                pj = psJ.tile([128, 256], F32, tag="psJ", name="proj_ps")
                                for e in range(8):
                                    nc.tensor.matmul(pj[:], wproj_sb[:, e, eb, :],
                                                     yTfull[:, e, po:po + 256],
                                                     start=(e == 0), stop=(e == 7))
                                nc.vector.tensor_add(x2T[:, eb, po:po + 256], pj[:],
                                                     xo_sb[:, eb, po:po + 256])
                            ln_transposed(h2T[:, :, po:po + 256],
                                          x2T[:, :, po:po + 256], 1,
                                          psP2, psB2, sqp2, rvp2, tmpl2, csz=256)

                # ------ phase E: MLP ----------
                with ExitStack() as pG:
                    h4Tp = pG.enter_context(tc.tile_pool(name="h4Tp", bufs=1))
                    h4T = h4Tp.tile([128, 32, TOKC], BF, tag="h4T")
                    with ExitStack() as pGa:
                        psH = pGa.enter_context(tc.tile_pool(name="psH", bufs=2, space="PSUM"))
                        wfcp = pGa.enter_context(tc.tile_pool(name="wfcp", bufs=3))
                        for m in range(32):
                            wfc_sb = wfcp.tile([128, 8, 128], BF, tag="wfc")
                            nc.sync.dma_start(wfc_sb[:], wfc_d[m, :, :, :])
                            h3 = psH.tile([128, 512], F32, tag="psH", name="h3")
                            for e in range(8):
                                nc.tensor.matmul(h3[:], wfc_sb[:, e, :], h2T[:, e, :],
                                                 start=(e == 0), stop=(e == 7))
                            nc.scalar.activation(h4T[:, m, :], h3[:],
                                                 AF.Gelu_apprx_tanh,
                                                 bias=bfcT_sb[:, m:m + 1])
                    with ExitStack() as pH:
                        psO = pH.enter_context(tc.tile_pool(name="psO", bufs=1, space="PSUM"))
                        wf2p = pH.enter_context(tc.tile_pool(name="wf2p", bufs=3))
                        outp = pH.enter_context(tc.tile_pool(name="outp", bufs=4))
                        y2 = [psO.tile([128, 512], F32, tag=f"y2_{eb}",
                                       name=f"y2_{eb}") for eb in range(8)]
                        for k in range(32):
                            wf2 = wf2p.tile([128, 8, 128], BF, tag="wf2")
                            nc.gpsimd.dma_start(wf2[:], wfc2_d[k, :, :, :])
                            for eb in range(8):
                                nc.tensor.matmul(y2[eb][:], wf2[:, eb, :],
                                                 h4T[:, k, :],
                                                 start=(k == 0), stop=(k == 31))
                        for eb in range(8):
                            o = outp.tile([128, TOKC], F32, tag="o")
                            nc.vector.tensor_add(o[:], y2[eb][:], x2T[:, eb, :])
                            nc.vector.tensor_scalar_add(o[:], o[:],
                                                        bfc2T_sb[:, eb:eb + 1])
                            nc.sync.dma_start(out_d[eb, :, :], o[:])

        for _rep in range(reps):
            emit_body()

    nc.compile()
    return nc


def prep_inputs(x, ln1_g, ln1_b, w_attn, b_attn, w_proj, b_proj,
                ln2_g, ln2_b, w_fc, b_fc, w_fc2, b_fc2):
    """Host-side prep: fold LN affine into weights, slice per core, cast bf16."""
    bf16 = ml_dtypes.bfloat16
    x = np.asarray(x, np.float32)
    w_attn = np.asarray(w_attn, np.float32)
    ln1_g = np.asarray(ln1_g, np.float32)
    ln1_b = np.asarray(ln1_b, np.float32)
    ln2_g = np.asarray(ln2_g, np.float32)
    ln2_b = np.asarray(ln2_b, np.float32)
    b_proj = np.asarray(b_proj, np.float32)
    wa_eff = ln1_g[:, None] * w_attn
    ba_eff = ln1_b @ w_attn + np.asarray(b_attn, np.float32)
    wf_eff = ln2_g[:, None] * np.asarray(w_fc, np.float32)
    bf_eff = ln2_b @ np.asarray(w_fc, np.float32) + np.asarray(b_fc, np.float32)

    tri = (np.arange(128)[None, :] >= np.arange(128)[:, None]).astype(bf16)
    wfc_arr = np.ascontiguousarray(
        wf_eff.reshape(8, 128, 32, 128).transpose(2, 1, 0, 3)).astype(bf16)
    bfcT = bf_eff.reshape(32, 128).T.astype(np.float32).copy()
    wfc2_arr = np.asarray(w_fc2, np.float32).reshape(32, 128, 8, 128).astype(bf16)
    bfc2T = np.asarray(b_fc2, np.float32).reshape(8, 128).T.copy()
    wproj_arr = np.ascontiguousarray(np.asarray(w_proj, np.float32).reshape(8, 128, 8, 128).transpose(1, 0, 2, 3)).astype(bf16)

    in_maps = []
    for r in range(N_CORES):
        b, sg = r // GSZ, r % GSZ
        cbase = COLS * sg
        qc_ = slice(cbase, cbase + COLS)
        kc_ = slice(E + cbase, E + cbase + COLS)
        vc_ = slice(2 * E + cbase, 2 * E + cbase + COLS)
        wq = np.ascontiguousarray(
            wa_eff[:, qc_].reshape(8, 128, 2, 128).transpose(1, 2, 0, 3)).astype(bf16)
        wk = np.ascontiguousarray(
            wa_eff[:, kc_].reshape(8, 128, 2, 128).transpose(1, 2, 0, 3)).astype(bf16)
        wv = np.ascontiguousarray(
            wa_eff[:, vc_].reshape(8, 128, 256).transpose(1, 0, 2)).astype(bf16)
        # owned tokens: blk=(p,bb): batch bb, qc=2p+(r%2), slice s=r//2
        par, s = r % 2, r // 2
        xov = np.concatenate(
            [x[bb, (2 * p + par) * 512 + s * 128: (2 * p + par) * 512 + (s + 1) * 128, :]
             for p in range(2) for bb in range(2)], axis=0) + b_proj  # [512, 1024]
        # -> transposed [128 e-in, 8 eb, 512 tok]
        xo = np.ascontiguousarray(
            xov.T.reshape(8, 128, TOKC).transpose(1, 0, 2)).astype(np.float32)
        xTh = np.ascontiguousarray(x[b].T.reshape(8, 128, T).transpose(1, 0, 2))
        in_maps.append({
            "xT": xTh.astype(bf16),
            "xo": xo,
            "wq": wq, "wk": wk, "wv": wv,
            "bq": np.ascontiguousarray(ba_eff[qc_].reshape(2, 128).T).astype(np.float32),
            "bk": np.ascontiguousarray(ba_eff[kc_].reshape(2, 128).T).astype(np.float32),
            "bvb": np.tile(ba_eff[vc_].astype(np.float32), (128, 1)),
            "wproj": wproj_arr,
            "trimask": tri,
            "wfc": wfc_arr, "bfcT": bfcT,
            "wfc2": wfc2_arr, "bfc2T": bfc2T,
        })
    return in_maps


def gather_output(results):
    out = np.empty((B, T, E), np.float32)
    for r in range(N_CORES):
        par, s = r % 2, r // 2
        o = results[r]["out"]          # [8 eb, 128 e, 512 (blk,t)]
        oc = o.reshape(8, 128, 4, 128).transpose(2, 3, 0, 1).reshape(4, 128, E)
        for blk in range(4):
            p, bb = blk // 2, blk % 2
            qc = 2 * p + par
            out[bb, qc * 512 + s * 128: qc * 512 + (s + 1) * 128, :] = oc[blk]
    return out


_CACHE = {}


def _get_runner():
    if "runner" in _CACHE:
        return _CACHE["runner"]
    import jax
    from jax.sharding import Mesh, PartitionSpec, NamedSharding
    from jax.experimental.shard_map import shard_map
    from concourse.bass2jax import _bass_exec_p, install_neuronx_cc_hook, partition_id_tensor

    nc = build_module()
    install_neuronx_cc_hook()
    partition_name = nc.partition_id_tensor.name if nc.partition_id_tensor else None
    in_names, out_names, out_avals = [], [], []
    for alloc in nc.m.functions[0].allocations:
        if not isinstance(alloc, mybir.MemoryLocationSet):
            continue
        name = alloc.memorylocations[0].name
        if alloc.kind == "ExternalInput":
            if name != partition_name:
                in_names.append(name)
        elif alloc.kind == "ExternalOutput":
            out_names.append(name)
            out_avals.append(jax.core.ShapedArray(
                tuple(alloc.tensor_shape), mybir.dt.np(alloc.dtype)))
    all_in = in_names + out_names + ([partition_name] if partition_name else [])

    def _body(*args):
        operands = list(args)
        if partition_name is not None:
            operands.append(partition_id_tensor())
        return tuple(_bass_exec_p.bind(
            *operands, out_avals=tuple(out_avals), in_names=tuple(all_in),
            out_names=tuple(out_names), lowering_input_output_aliases=(),
            sim_require_finite=True, sim_require_nnan=True, nc=nc))

    devices = jax.devices()[:N_CORES]
    mesh = Mesh(np.asarray(devices), ("core",))
    n_io = len(in_names) + len(out_names)
    fn = jax.jit(
        shard_map(_body, mesh=mesh, in_specs=(PartitionSpec("core"),) * n_io,
                  out_specs=(PartitionSpec("core"),) * len(out_names),
                  check_rep=False),
        keep_unused=True)
    sharding = NamedSharding(mesh, PartitionSpec("core"))
    _CACHE["runner"] = (fn, in_names, out_names, out_avals, sharding)
    return _CACHE["runner"]


def run_device(in_maps):
    import jax
    fn, in_names, out_names, out_avals, sharding = _get_runner()
    concat_in = [
        np.concatenate([np.asarray(in_maps[c][n]) for c in range(N_CORES)], axis=0)
        for n in in_names]
    concat_zero = [np.zeros((N_CORES * a.shape[0], *a.shape[1:]), a.dtype)
                   for a in out_avals]
    args = [jax.device_put(a, sharding) for a in concat_in + concat_zero]
    outs = fn(*args)
    jax.block_until_ready(outs)
    return [
        {n: np.asarray(outs[i]).reshape(N_CORES, *out_avals[i].shape)[c]
         for i, n in enumerate(out_names)}
        for c in range(N_CORES)], args, fn


def kernel(**inputs):
    in_maps = prep_inputs(**inputs)
    results, _, _ = run_device(in_maps)
    return gather_output(results).astype(np.float32)
